# revision 6
# baseline (speedup 1.0000x reference)
"""Trainium2 Bass kernel: decoder-only GQA attention, single decode step.

Problem shapes (hardcoded): B=16, H=32 query heads, KVH=8 KV heads,
HD=128, S=4096, D=4096, Q=1.

Sharding: tensor-parallel over heads across 8 NeuronCores. Core c owns
KV head c and query heads [4c, 4c+4): Wq/Wk/Wv column-split, Wo
row-split; the Wo partial products ([B, D] per core) are summed on the
host. KV cache is sharded by KV head; each core streams its 33.5MB K
and V shards once, copies them to the new_key/new_value outputs, and
computes attention from the same SBUF tiles.

Device-side layout trick: the host uploads K pre-transposed per batch
([B, HD, S] "d-major") so the scores matmul can consume K chunks
directly as the stationary operand (out = K_chunk @ qT -> scoresT
[s, h]) with zero on-device transposes. new_key is returned by the
device in the same d-major layout and transposed back on the host.
Softmax runs in the scoresT layout: exp on ScalarE, causal masking via
two memsets (step positions are host-known and baked into the
program), the denominator via a ones-vector matmul on TensorE, and the
normalization is applied to the tiny [4, 128] attention output.
"""

import math

import numpy as np

import concourse.bass as bass
import concourse.mybir as mybir
import concourse.tile as tile
from concourse import bacc
from concourse.bass_utils import run_bass_kernel_spmd
from concourse.masks import make_identity

B = 16
H = 32
KVH = 8
HD = 128
S = 4096
D = 4096
NCORES = 8
G = H // KVH  # query heads per KV head / per core
SC = S // 128  # 128-row s-chunks per batch

F32 = mybir.dt.float32

_cache: dict = {}
LAST_RESULT = None


def _build_program(steps):
    """Build + compile the per-core SPMD program; steps values are baked in."""
    nc = bacc.Bacc("TRN2", target_bir_lowering=False, debug=False, num_devices=NCORES)

    kt_d = nc.dram_tensor("kt", [B, HD, S], F32, kind="ExternalInput").ap()
    v_d = nc.dram_tensor("v", [B, S, HD], F32, kind="ExternalInput").ap()
    hst_d = nc.dram_tensor("hst", [D, B], F32, kind="ExternalInput").ap()
    wq_d = nc.dram_tensor("wq", [D, G * HD], F32, kind="ExternalInput").ap()
    wk_d = nc.dram_tensor("wk", [D, HD], F32, kind="ExternalInput").ap()
    wv_d = nc.dram_tensor("wv", [D, HD], F32, kind="ExternalInput").ap()
    wo_d = nc.dram_tensor("wo", [G * HD, D], F32, kind="ExternalInput").ap()
    cosq_d = nc.dram_tensor("cosq", [B, G * HD], F32, kind="ExternalInput").ap()
    sinq_d = nc.dram_tensor("sinq", [B, G * HD], F32, kind="ExternalInput").ap()
    cosk_d = nc.dram_tensor("cosk", [B, HD], F32, kind="ExternalInput").ap()
    sink_d = nc.dram_tensor("sink", [B, HD], F32, kind="ExternalInput").ap()
    pmask_d = nc.dram_tensor("pmask", [128, B], F32, kind="ExternalInput").ap()

    nkt_d = nc.dram_tensor("new_kt", [B, HD, S], F32, kind="ExternalOutput").ap()
    nv_d = nc.dram_tensor("new_v", [B, S, HD], F32, kind="ExternalOutput").ap()
    ao_d = nc.dram_tensor("attn_part", [B, D], F32, kind="ExternalOutput").ap()

    inv_sqrt_hd = 1.0 / math.sqrt(HD)

    with tile.TileContext(nc) as tc:
        with tc.tile_pool(name="const", bufs=1) as cpool:
            ident = cpool.tile([128, 128], F32)
            make_identity(nc, ident[:])
            ones = cpool.tile([128, 1], F32)
            nc.vector.memset(ones[:], 1.0)

            hst_s = cpool.tile([128, SC * B], F32)
            nc.sync.dma_start(
                hst_s[:].rearrange("p (c b) -> p c b", b=B),
                hst_d.rearrange("(c p) b -> p c b", p=128),
            )
            cq = cpool.tile([B, G * HD], F32)
            nc.sync.dma_start(cq[:], cosq_d)
            sq = cpool.tile([B, G * HD], F32)
            nc.sync.dma_start(sq[:], sinq_d)
            ck = cpool.tile([B, HD], F32)
            nc.sync.dma_start(ck[:], cosk_d)
            sk = cpool.tile([B, HD], F32)
            nc.sync.dma_start(sk[:], sink_d)
            pmask = cpool.tile([128, B], F32)
            nc.sync.dma_start(pmask[:], pmask_d)

            # ---- projections: q = hs@Wq, k = hs@Wk, v = hs@Wv ----
            qT_s = cpool.tile([128, G * B], F32)  # [d, (h,b)]
            kT_s = cpool.tile([128, B], F32)  # [d, b]
            vrow_s = cpool.tile([B, HD], F32)  # [b, d]
            oT_s = cpool.tile([128, G * B], F32)  # [d, (h,b)] attn out

            with (
                tc.tile_pool(name="wstream", bufs=3) as wpool,
                tc.tile_pool(name="ppsum", bufs=1, space="PSUM") as ppsum,
                tc.tile_pool(name="rope", bufs=1) as rpool,
            ):
                qp = ppsum.tile([B, G * HD], F32)
                kp = ppsum.tile([B, HD], F32)
                vp = ppsum.tile([B, HD], F32)
                for ci in range(SC):
                    wq_t = wpool.tile([128, G * HD], F32, tag="wq")
                    nc.sync.dma_start(wq_t[:], wq_d[ci * 128 : (ci + 1) * 128, :])
                    wk_t = wpool.tile([128, HD], F32, tag="wk")
                    nc.sync.dma_start(wk_t[:], wk_d[ci * 128 : (ci + 1) * 128, :])
                    wv_t = wpool.tile([128, HD], F32, tag="wv")
                    nc.sync.dma_start(wv_t[:], wv_d[ci * 128 : (ci + 1) * 128, :])
                    lhs = hst_s[:, ci * B : (ci + 1) * B]
                    st, sp = (ci == 0), (ci == SC - 1)
                    nc.tensor.matmul(qp[:], lhs, wq_t[:], start=st, stop=sp)
                    nc.tensor.matmul(kp[:], lhs, wk_t[:], start=st, stop=sp)
                    nc.tensor.matmul(vp[:], lhs, wv_t[:], start=st, stop=sp)

                # ---- RoPE: x*cos + rotate_half(x)*sin ----
                # sinq/sink arrive sign-folded: first halves hold -sin.
                def rope(dst, src_psum, cos_t, sin_t, nh):
                    tmp = rpool.tile([B, nh * HD], F32, tag="rtmp")
                    xc = rpool.tile([B, nh * HD], F32, tag="rxc")

                    def hv(ap):
                        r = ap.rearrange("b (h two f) -> b h two f", two=2, f=64)
                        return r[:, :, 0, :], r[:, :, 1, :]

                    xf, xs = hv(src_psum[:])
                    tf, ts_ = hv(tmp[:])
                    sf, ss = hv(sin_t)
                    nc.vector.tensor_mul(tf, xs, sf)
                    nc.vector.tensor_mul(ts_, xf, ss)
                    nc.vector.tensor_mul(xc[:], src_psum[:], cos_t)
                    nc.vector.tensor_add(dst, xc[:], tmp[:])

                q_rope = rpool.tile([B, G * HD], F32)
                rope(q_rope[:], qp, cq[:], sq[:], G)
                k_rope = rpool.tile([B, HD], F32)
                rope(k_rope[:], kp, ck[:], sk[:], 1)
                nc.vector.tensor_copy(vrow_s[:], vp[:])

                # ---- transpose q_rope/k_rope to [d, b] layouts ----
                qT_p = ppsum.tile([128, G * B], F32)
                for h in range(G):
                    nc.tensor.transpose(
                        qT_p[:, h * B : (h + 1) * B],
                        q_rope[:, h * HD : (h + 1) * HD],
                        ident[:B, :B],
                    )
                kT_p = ppsum.tile([128, B], F32)
                nc.tensor.transpose(kT_p[:], k_rope[:], ident[:B, :B])
                nc.vector.tensor_copy(qT_s[:], qT_p[:])
                nc.vector.tensor_copy(kT_s[:], kT_p[:])

            # ---- per-batch attention + cache copy/update ----
            with (
                tc.tile_pool(name="kv", bufs=3) as kvpool,
                tc.tile_pool(name="work", bufs=3) as work,
                tc.tile_pool(name="scp", bufs=2, space="PSUM") as sc_psum,
                tc.tile_pool(name="pvp", bufs=2, space="PSUM") as pv_psum,
                tc.tile_pool(name="smp", bufs=1, space="PSUM") as sm_psum,
                tc.tile_pool(name="otp", bufs=1, space="PSUM") as ot_psum,
            ):
                for b in range(B):
                    step = int(steps[b])
                    sci, spi = step // 128, step % 128

                    kt_t = kvpool.tile([128, S], F32, tag="kt")
                    nc.sync.dma_start(kt_t[:, : S // 2], kt_d[b][:, : S // 2])
                    nc.sync.dma_start(kt_t[:, S // 2 :], kt_d[b][:, S // 2 :])
                    v_t = kvpool.tile([128, S], F32, tag="v")
                    v_src = v_d[b].rearrange("(c p) d -> p c d", p=128)
                    v_dst = v_t[:].rearrange("p (c d) -> p c d", d=HD)
                    nc.sync.dma_start(v_dst[:, : SC // 2], v_src[:, : SC // 2])
                    nc.sync.dma_start(v_dst[:, SC // 2 :], v_src[:, SC // 2 :])

                    # scatter new k/v row at current step
                    nc.vector.tensor_copy(
                        kt_t[:, step : step + 1], kT_s[:, b : b + 1]
                    )
                    nc.sync.dma_start(
                        v_t[spi : spi + 1, sci * HD : (sci + 1) * HD],
                        vrow_s[b : b + 1, :],
                    )

                    # copy updated cache shard to outputs
                    nc.sync.dma_start(nkt_d[b][:, : S // 2], kt_t[:, : S // 2])
                    nc.sync.dma_start(nkt_d[b][:, S // 2 :], kt_t[:, S // 2 :])
                    nv_dst = nv_d[b].rearrange("(c p) d -> p c d", p=128)
                    nc.sync.dma_start(nv_dst[:, : SC // 2], v_dst[:, : SC // 2])
                    nc.sync.dma_start(nv_dst[:, SC // 2 :], v_dst[:, SC // 2 :])

                    # scoresT[s, h] = K[s, :] @ qT[:, h] per 128-row chunk
                    qT_b = qT_s[:].rearrange("p (h b) -> p b h", b=B)[:, b, :]
                    scp = sc_psum.tile([128, SC * G], F32)
                    for c in range(SC):
                        nc.tensor.matmul(
                            scp[:, c * G : (c + 1) * G],
                            kt_t[:, c * 128 : (c + 1) * 128],
                            qT_b,
                            start=True,
                            stop=True,
                        )

                    expm = work.tile([128, SC * G], F32, tag="expm")
                    nc.scalar.activation(
                        expm[:], scp[:], mybir.ActivationFunctionType.Exp,
                        scale=inv_sqrt_hd,
                    )
                    # causal mask: zero probs for s > step (baked positions)
                    if sci < SC - 1:
                        nc.vector.memset(expm[:, (sci + 1) * G :], 0.0)
                    if spi < 127:
                        sl = expm[:, sci * G : (sci + 1) * G]
                        nc.vector.tensor_scalar_mul(sl, sl, pmask[:, b : b + 1])

                    pvp = pv_psum.tile([G, HD], F32)
                    smp = sm_psum.tile([G, 1], F32)
                    for c in range(SC):
                        lhs = expm[:, c * G : (c + 1) * G]
                        st, sp = (c == 0), (c == SC - 1)
                        nc.tensor.matmul(
                            pvp[:], lhs, v_t[:, c * 128 : (c + 1) * 128],
                            start=st, stop=sp,
                        )
                        nc.tensor.matmul(smp[:], lhs, ones[:], start=st, stop=sp)

                    rec = work.tile([G, 1], F32, tag="rec")
                    nc.vector.reciprocal(rec[:], smp[:])
                    osc = work.tile([G, HD], F32, tag="osc")
                    nc.vector.tensor_scalar_mul(osc[:], pvp[:], rec[:])
                    otp = ot_psum.tile([128, G], F32)
                    nc.tensor.transpose(otp[:], osc[:], ident[:G, :G])
                    oT_b = oT_s[:].rearrange("p (h b) -> p b h", b=B)[:, b, :]
                    nc.vector.tensor_copy(oT_b, otp[:])

                # ---- output projection: attn_part = out_flat @ Wo_shard ----
                with (
                    tc.tile_pool(name="wo", bufs=3) as wopool,
                    tc.tile_pool(name="wops", bufs=2, space="PSUM") as wops,
                ):
                    NT = 512
                    for n in range(D // NT):
                        wp = wops.tile([B, NT], F32)
                        for r in range(G):
                            wo_t = wopool.tile([128, NT], F32, tag="wo")
                            nc.sync.dma_start(
                                wo_t[:],
                                wo_d[r * 128 : (r + 1) * 128, n * NT : (n + 1) * NT],
                            )
                            nc.tensor.matmul(
                                wp[:], oT_s[:, r * B : (r + 1) * B], wo_t[:],
                                start=(r == 0), stop=(r == G - 1),
                            )
                        res_t = wopool.tile([B, NT], F32, tag="res")
                        nc.vector.tensor_copy(res_t[:], wp[:])
                        nc.sync.dma_start(ao_d[:, n * NT : (n + 1) * NT], res_t[:])

    nc.compile()
    return nc


def kernel(hidden_states, attention_mask, current_steps, cos, sin,
           past_key, past_value, Wq, Wk, Wv, Wo):
    global LAST_RESULT
    f32 = np.float32
    hs = np.ascontiguousarray(np.asarray(hidden_states, dtype=f32).reshape(B, D))
    steps = np.asarray(current_steps).astype(np.int64).reshape(B)
    cosv = np.asarray(cos, dtype=f32).reshape(B, HD)
    sinv = np.asarray(sin, dtype=f32).reshape(B, HD)
    pk = np.asarray(past_key, dtype=f32)
    pv = np.asarray(past_value, dtype=f32)
    Wq = np.asarray(Wq, dtype=f32)
    Wk = np.asarray(Wk, dtype=f32)
    Wv = np.asarray(Wv, dtype=f32)
    Wo = np.asarray(Wo, dtype=f32)

    key = tuple(steps.tolist())
    nc = _cache.get(key)
    if nc is None:
        _cache.clear()
        nc = _build_program(steps)
        _cache[key] = nc

    hst = np.ascontiguousarray(hs.T)  # [D, B]
    # sign-folded rotate-half sin: first half -sin, second half +sin
    sin_rot = np.concatenate([-sinv[:, : HD // 2], sinv[:, HD // 2 :]], axis=1)
    cosq = np.ascontiguousarray(np.tile(cosv, (1, G)))
    sinq = np.ascontiguousarray(np.tile(sin_rot, (1, G)))

    # per-partition causal mask column for the partial s-chunk at each step
    pmask_np = (np.arange(128)[:, None] <= (steps % 128)[None, :]).astype(f32)
    pmask_np = np.ascontiguousarray(pmask_np)

    # d-major (transposed) K cache: [B, KVH, HD, S]
    pkT = np.ascontiguousarray(pk.transpose(0, 1, 3, 2))

    in_maps = []
    for c in range(NCORES):
        in_maps.append({
            "kt": np.ascontiguousarray(pkT[:, c]),
            "v": np.ascontiguousarray(pv[:, c]),
            "hst": hst,
            "wq": np.ascontiguousarray(Wq[:, c * G * HD : (c + 1) * G * HD]),
            "wk": np.ascontiguousarray(Wk[:, c * HD : (c + 1) * HD]),
            "wv": np.ascontiguousarray(Wv[:, c * HD : (c + 1) * HD]),
            "wo": np.ascontiguousarray(Wo[c * G * HD : (c + 1) * G * HD, :]),
            "cosq": cosq,
            "sinq": sinq,
            "cosk": cosv,
            "sink": np.ascontiguousarray(sin_rot),
            "pmask": pmask_np,
        })

    res = run_bass_kernel_spmd(nc, in_maps, list(range(NCORES)))
    LAST_RESULT = res

    attn_out = np.zeros((B, D), dtype=f32)
    new_key = np.empty((B, KVH, S, HD), dtype=f32)
    new_value = np.empty((B, KVH, S, HD), dtype=f32)
    for c in range(NCORES):
        r = res.results[c]
        attn_out += r["attn_part"]
        new_key[:, c] = r["new_kt"].transpose(0, 2, 1)
        new_value[:, c] = r["new_v"]
    return attn_out.reshape(B, 1, D), new_key, new_value


# revision 16
# speedup vs baseline: 1.1824x; 1.1824x over previous
"""Trainium2 Bass kernel: decoder-only GQA attention, single decode step.

Problem shapes (hardcoded): B=16, H=32 query heads, KVH=8 KV heads,
HD=128, S=4096, D=4096, Q=1.

Sharding: tensor-parallel over heads across 8 NeuronCores. Core c owns
KV head c and query heads [4c, 4c+4): Wq/Wk/Wv column-split, Wo
row-split; the Wo partial products ([B, D] per core) are summed on the
host. KV cache is sharded by KV head; each core streams its 33.5MB K
and V shards once, copies them to the new_key/new_value outputs, and
computes attention from the same SBUF tiles.

Device-side layout trick: the host uploads K pre-transposed per batch
([B, HD, S] "d-major") so the scores matmul can consume K chunks
directly as the stationary operand (out = K_chunk @ qT -> scoresT
[s, h]) with zero on-device transposes. new_key is returned by the
device in the same d-major layout and transposed back on the host.
Softmax runs in the scoresT layout: exp on ScalarE, causal masking via
two memsets (step positions are host-known and baked into the
program), the denominator via a ones-vector matmul on TensorE, and the
normalization is applied to the tiny [4, 128] attention output.
"""

import math

import numpy as np

import concourse.bass as bass
import concourse.mybir as mybir
import concourse.tile as tile
from concourse import bacc
from concourse.bass_utils import run_bass_kernel_spmd
from concourse.masks import make_identity

B = 16
H = 32
KVH = 8
HD = 128
S = 4096
D = 4096
NCORES = 8
G = H // KVH  # query heads per KV head / per core
SC = S // 128  # 128-row s-chunks per batch
VW = HD + 1  # V chunk width incl. baked ones column

F32 = mybir.dt.float32

_cache: dict = {}
LAST_RESULT = None


def _build_program(steps):
    """Build + compile the per-core SPMD program; steps values are baked in."""
    nc = bacc.Bacc("TRN2", target_bir_lowering=False, debug=False, num_devices=NCORES)

    # v arrives pre-swizzled to the SBUF layout [128, SC*(HD+1)]: partition
    # p holds V[c*128+p, :] at free offset c*(HD+1), with a constant-1.0
    # column at c*(HD+1)+HD (so the PV matmul's rhs [128, HD+1] also
    # produces the softmax denominator in its last column).
    kt_d = nc.dram_tensor("kt", [B, HD, S], F32, kind="ExternalInput").ap()
    v_d = nc.dram_tensor("v", [B, 128, SC * VW], F32, kind="ExternalInput").ap()
    hst_d = nc.dram_tensor("hst", [128, SC * B], F32, kind="ExternalInput").ap()
    wq_d = nc.dram_tensor("wq", [D, G * HD], F32, kind="ExternalInput").ap()
    wk_d = nc.dram_tensor("wk", [D, HD], F32, kind="ExternalInput").ap()
    wv_d = nc.dram_tensor("wv", [D, HD], F32, kind="ExternalInput").ap()
    wo_d = nc.dram_tensor("wo", [G * HD, D], F32, kind="ExternalInput").ap()
    cosq_d = nc.dram_tensor("cosq", [B, G * HD], F32, kind="ExternalInput").ap()
    sinq_d = nc.dram_tensor("sinq", [B, G * HD], F32, kind="ExternalInput").ap()
    cosk_d = nc.dram_tensor("cosk", [B, HD], F32, kind="ExternalInput").ap()
    sink_d = nc.dram_tensor("sink", [B, HD], F32, kind="ExternalInput").ap()
    pmask_d = nc.dram_tensor("pmask", [128, B], F32, kind="ExternalInput").ap()

    nkt_d = nc.dram_tensor("new_kt", [B, HD, S], F32, kind="ExternalOutput").ap()
    nv_d = nc.dram_tensor("new_v", [B, 128, SC * VW], F32, kind="ExternalOutput").ap()
    ao_d = nc.dram_tensor("attn_part", [B, D], F32, kind="ExternalOutput").ap()

    inv_sqrt_hd = 1.0 / math.sqrt(HD)

    with tile.TileContext(nc) as tc:
        with tc.tile_pool(name="const", bufs=1) as cpool:
            ident = cpool.tile([128, 128], F32)
            make_identity(nc, ident[:])

            hst_s = cpool.tile([128, SC * B], F32)
            nc.sync.dma_start(hst_s[:], hst_d)
            cq = cpool.tile([B, G * HD], F32)
            nc.sync.dma_start(cq[:], cosq_d)
            sq = cpool.tile([B, G * HD], F32)
            nc.sync.dma_start(sq[:], sinq_d)
            ck = cpool.tile([B, HD], F32)
            nc.sync.dma_start(ck[:], cosk_d)
            sk = cpool.tile([B, HD], F32)
            nc.sync.dma_start(sk[:], sink_d)
            pmask = cpool.tile([128, B], F32)
            nc.sync.dma_start(pmask[:], pmask_d)

            # ---- projections: q = hs@Wq, k = hs@Wk, v = hs@Wv ----
            qT_s = cpool.tile([128, G * B], F32)  # [d, (h,b)]
            kT_s = cpool.tile([128, B], F32)  # [d, b]
            vrow_s = cpool.tile([B, HD], F32)  # [b, d]
            oT_s = cpool.tile([128, G * B], F32)  # [d, (h,b)] attn out

            with (
                tc.tile_pool(name="wstream", bufs=3) as wpool,
                tc.tile_pool(name="ppsum", bufs=1, space="PSUM") as ppsum,
                tc.tile_pool(name="rope", bufs=1) as rpool,
            ):
                qp = ppsum.tile([B, G * HD], F32)
                kp = ppsum.tile([B, HD], F32)
                vp = ppsum.tile([B, HD], F32)
                for ci in range(SC):
                    wq_t = wpool.tile([128, G * HD], F32, tag="wq")
                    nc.sync.dma_start(wq_t[:], wq_d[ci * 128 : (ci + 1) * 128, :])
                    wk_t = wpool.tile([128, HD], F32, tag="wk")
                    nc.sync.dma_start(wk_t[:], wk_d[ci * 128 : (ci + 1) * 128, :])
                    wv_t = wpool.tile([128, HD], F32, tag="wv")
                    nc.sync.dma_start(wv_t[:], wv_d[ci * 128 : (ci + 1) * 128, :])
                    lhs = hst_s[:, ci * B : (ci + 1) * B]
                    st, sp = (ci == 0), (ci == SC - 1)
                    nc.tensor.matmul(qp[:], lhs, wq_t[:], start=st, stop=sp)
                    nc.tensor.matmul(kp[:], lhs, wk_t[:], start=st, stop=sp)
                    nc.tensor.matmul(vp[:], lhs, wv_t[:], start=st, stop=sp)

                # ---- RoPE: x*cos + rotate_half(x)*sin ----
                # sinq/sink arrive sign-folded: first halves hold -sin.
                def rope(dst, src_psum, cos_t, sin_t, nh):
                    tmp = rpool.tile([B, nh * HD], F32, tag="rtmp")
                    xc = rpool.tile([B, nh * HD], F32, tag="rxc")

                    def hv(ap):
                        r = ap.rearrange("b (h two f) -> b h two f", two=2, f=64)
                        return r[:, :, 0, :], r[:, :, 1, :]

                    xf, xs = hv(src_psum[:])
                    tf, ts_ = hv(tmp[:])
                    sf, ss = hv(sin_t)
                    nc.vector.tensor_mul(tf, xs, sf)
                    nc.vector.tensor_mul(ts_, xf, ss)
                    nc.vector.tensor_mul(xc[:], src_psum[:], cos_t)
                    nc.vector.tensor_add(dst, xc[:], tmp[:])

                q_rope = rpool.tile([B, G * HD], F32)
                rope(q_rope[:], qp, cq[:], sq[:], G)
                k_rope = rpool.tile([B, HD], F32)
                rope(k_rope[:], kp, ck[:], sk[:], 1)
                nc.vector.tensor_copy(vrow_s[:], vp[:])

                # ---- transpose q_rope/k_rope to [d, b] layouts ----
                qT_p = ppsum.tile([128, G * B], F32)
                for h in range(G):
                    nc.tensor.transpose(
                        qT_p[:, h * B : (h + 1) * B],
                        q_rope[:, h * HD : (h + 1) * HD],
                        ident[:B, :B],
                    )
                kT_p = ppsum.tile([128, B], F32)
                nc.tensor.transpose(kT_p[:], k_rope[:], ident[:B, :B])
                nc.vector.tensor_copy(qT_s[:], qT_p[:])
                nc.vector.tensor_copy(kT_s[:], kT_p[:])

            # ---- per-batch attention + cache copy/update ----
            with (
                tc.tile_pool(name="kv", bufs=3) as kvpool,
                tc.tile_pool(name="work", bufs=3) as work,
                tc.tile_pool(name="scp", bufs=2, space="PSUM") as sc_psum,
                tc.tile_pool(name="pvp", bufs=2, space="PSUM") as pv_psum,
                tc.tile_pool(name="otp", bufs=2, space="PSUM") as ot_psum,
            ):
                for b in range(B):
                    step = int(steps[b])
                    sci, spi = step // 128, step % 128

                    kt_t = kvpool.tile([128, S], F32, tag="kt")
                    nc.sync.dma_start(kt_t[:, : S // 2], kt_d[b][:, : S // 2])
                    nc.sync.dma_start(kt_t[:, S // 2 :], kt_d[b][:, S // 2 :])
                    VF = SC * VW
                    v_t = kvpool.tile([128, VF], F32, tag="v")
                    nc.sync.dma_start(v_t[:, : VF // 2], v_d[b][:, : VF // 2])
                    nc.sync.dma_start(v_t[:, VF // 2 :], v_d[b][:, VF // 2 :])

                    # scatter new k/v row at current step
                    nc.vector.tensor_copy(
                        kt_t[:, step : step + 1], kT_s[:, b : b + 1]
                    )
                    nc.sync.dma_start(
                        v_t[spi : spi + 1, sci * VW : sci * VW + HD],
                        vrow_s[b : b + 1, :],
                    )

                    # copy updated cache shard to outputs
                    nc.sync.dma_start(nkt_d[b][:, : S // 2], kt_t[:, : S // 2])
                    nc.sync.dma_start(nkt_d[b][:, S // 2 :], kt_t[:, S // 2 :])
                    nc.sync.dma_start(nv_d[b][:, : VF // 2], v_t[:, : VF // 2])
                    nc.sync.dma_start(nv_d[b][:, VF // 2 :], v_t[:, VF // 2 :])

                    # scoresT[s, h] = K[s, :] @ qT[:, h] per 128-row chunk
                    qT_b = qT_s[:].rearrange("p (h b) -> p b h", b=B)[:, b, :]
                    scp = sc_psum.tile([128, SC * G], F32)
                    for c in range(SC):
                        nc.tensor.matmul(
                            scp[:, c * G : (c + 1) * G],
                            kt_t[:, c * 128 : (c + 1) * 128],
                            qT_b,
                            start=True,
                            stop=True,
                        )

                    expm = work.tile([128, SC * G], F32, tag="expm")
                    nc.scalar.activation(
                        expm[:], scp[:], mybir.ActivationFunctionType.Exp,
                        scale=inv_sqrt_hd,
                    )
                    # causal mask: zero probs for s > step (baked positions)
                    if sci < SC - 1:
                        nc.vector.memset(expm[:, (sci + 1) * G :], 0.0)
                    if spi < 127:
                        sl = expm[:, sci * G : (sci + 1) * G]
                        nc.vector.tensor_scalar_mul(sl, sl, pmask[:, b : b + 1])

                    # PV matmul; rhs column HD is the baked ones column, so
                    # pvp[:, HD] accumulates the softmax denominator.
                    pvp = pv_psum.tile([G, VW], F32)
                    for c in range(SC):
                        nc.tensor.matmul(
                            pvp[:], expm[:, c * G : (c + 1) * G],
                            v_t[:, c * VW : (c + 1) * VW],
                            start=(c == 0), stop=(c == SC - 1),
                        )

                    rec = work.tile([G, 1], F32, tag="rec")
                    nc.vector.reciprocal(rec[:], pvp[:, HD : HD + 1])
                    osc = work.tile([G, HD], F32, tag="osc")
                    nc.vector.tensor_scalar_mul(osc[:], pvp[:, :HD], rec[:])
                    otp = ot_psum.tile([128, G], F32)
                    nc.tensor.transpose(otp[:], osc[:], ident[:G, :G])
                    oT_b = oT_s[:].rearrange("p (h b) -> p b h", b=B)[:, b, :]
                    nc.vector.tensor_copy(oT_b, otp[:])

                # ---- output projection: attn_part = out_flat @ Wo_shard ----
                with (
                    tc.tile_pool(name="wo", bufs=3) as wopool,
                    tc.tile_pool(name="wops", bufs=2, space="PSUM") as wops,
                ):
                    NT = 512
                    for n in range(D // NT):
                        wp = wops.tile([B, NT], F32)
                        for r in range(G):
                            wo_t = wopool.tile([128, NT], F32, tag="wo")
                            nc.sync.dma_start(
                                wo_t[:],
                                wo_d[r * 128 : (r + 1) * 128, n * NT : (n + 1) * NT],
                            )
                            nc.tensor.matmul(
                                wp[:], oT_s[:, r * B : (r + 1) * B], wo_t[:],
                                start=(r == 0), stop=(r == G - 1),
                            )
                        res_t = wopool.tile([B, NT], F32, tag="res")
                        nc.vector.tensor_copy(res_t[:], wp[:])
                        nc.sync.dma_start(ao_d[:, n * NT : (n + 1) * NT], res_t[:])

    nc.compile()
    return nc


def kernel(hidden_states, attention_mask, current_steps, cos, sin,
           past_key, past_value, Wq, Wk, Wv, Wo):
    global LAST_RESULT
    f32 = np.float32
    hs = np.ascontiguousarray(np.asarray(hidden_states, dtype=f32).reshape(B, D))
    steps = np.asarray(current_steps).astype(np.int64).reshape(B)
    cosv = np.asarray(cos, dtype=f32).reshape(B, HD)
    sinv = np.asarray(sin, dtype=f32).reshape(B, HD)
    pk = np.asarray(past_key, dtype=f32)
    pv = np.asarray(past_value, dtype=f32)
    Wq = np.asarray(Wq, dtype=f32)
    Wk = np.asarray(Wk, dtype=f32)
    Wv = np.asarray(Wv, dtype=f32)
    Wo = np.asarray(Wo, dtype=f32)

    key = tuple(steps.tolist())
    nc = _cache.get(key)
    if nc is None:
        _cache.clear()
        nc = _build_program(steps)
        _cache[key] = nc

    # hs.T pre-swizzled to SBUF layout: [128, (chunk, batch)]
    hst = np.ascontiguousarray(
        hs.reshape(B, SC, 128).transpose(2, 1, 0).reshape(128, SC * B)
    )
    # sign-folded rotate-half sin: first half -sin, second half +sin
    sin_rot = np.concatenate([-sinv[:, : HD // 2], sinv[:, HD // 2 :]], axis=1)
    cosq = np.ascontiguousarray(np.tile(cosv, (1, G)))
    sinq = np.ascontiguousarray(np.tile(sin_rot, (1, G)))

    # per-partition causal mask column for the partial s-chunk at each step
    pmask_np = (np.arange(128)[:, None] <= (steps % 128)[None, :]).astype(f32)
    pmask_np = np.ascontiguousarray(pmask_np)

    # d-major (transposed) K cache: [B, KVH, HD, S]
    pkT = np.ascontiguousarray(pk.transpose(0, 1, 3, 2))
    # V pre-swizzled to SBUF layout with baked ones columns:
    # vsw[b, p, c*VW + d] = V[b, c*128 + p, d]; vsw[b, p, c*VW + HD] = 1.0
    vsw = np.empty((B, KVH, 128, SC, VW), dtype=f32)
    vsw[..., HD] = 1.0
    vsw[..., :HD] = pv.reshape(B, KVH, SC, 128, HD).transpose(0, 1, 3, 2, 4)
    vsw = vsw.reshape(B, KVH, 128, SC * VW)

    in_maps = []
    for c in range(NCORES):
        in_maps.append({
            "kt": np.ascontiguousarray(pkT[:, c]),
            "v": np.ascontiguousarray(vsw[:, c]),
            "hst": hst,
            "wq": np.ascontiguousarray(Wq[:, c * G * HD : (c + 1) * G * HD]),
            "wk": np.ascontiguousarray(Wk[:, c * HD : (c + 1) * HD]),
            "wv": np.ascontiguousarray(Wv[:, c * HD : (c + 1) * HD]),
            "wo": np.ascontiguousarray(Wo[c * G * HD : (c + 1) * G * HD, :]),
            "cosq": cosq,
            "sinq": sinq,
            "cosk": cosv,
            "sink": np.ascontiguousarray(sin_rot),
            "pmask": pmask_np,
        })

    res = run_bass_kernel_spmd(nc, in_maps, list(range(NCORES)))
    LAST_RESULT = res

    attn_out = np.zeros((B, D), dtype=f32)
    new_key = np.empty((B, KVH, S, HD), dtype=f32)
    new_value = np.empty((B, KVH, S, HD), dtype=f32)
    for c in range(NCORES):
        r = res.results[c]
        attn_out += r["attn_part"]
        new_key[:, c] = r["new_kt"].transpose(0, 2, 1)
        nv = r["new_v"].reshape(B, 128, SC, VW)[..., :HD]
        new_value[:, c] = nv.transpose(0, 2, 1, 3).reshape(B, S, HD)
    return attn_out.reshape(B, 1, D), new_key, new_value
